# revision 10
# baseline (speedup 1.0000x reference)
"""Grouped per-channel Linear + ReLU on 8 TRN2 NeuronCores.

Problem: out[b,c,e] = relu(sum_s x[b,s,c] * W[c,s,e] + bias[c,e])
  x: (256, 2048, 32) f32, W: (32, 2048, 2048) f32, bias: (32, 2048) f32
  out: (256, 32, 2048) f32

Sharding: expert/channel parallel - core i computes channels [4i, 4i+4).
Each core runs 4 independent GEMMs of (256x2048)@(2048x2048) with the
contraction dim S on SBUF partitions; x is host-transposed to
[CPC, P, KT, B] fp16 so DMA descriptors are >=2 KB contiguous runs.

W is quantized host-side to float8e3 (E3M4: 4 mantissa bits, scaled to
max 15.0; W is U(-b,b) so rms rel l2 error ~1.2e-2, under the 2e-2
gate). fp8 at normal (non-DoubleRow) rate streams into the PE at the
same N cycles/matmul as fp16, so the PE floor is unchanged - but W's
HBM traffic is 1 B/elem (16.8 MB/core) AND, unlike the int8 scheme,
needs NO on-chip dequant: no SWDGE cast stream, no DVE tensor_copy, no
dequant dependency chains. The PE reads the fp8 bytes directly as the
moving operand (mixed-dtype matmul fp16 lhsT x fp8e3 rhs verified
bit-exact on HW). DMA rings are statically split: W rides the sync ring
alone (16.8 MB, in k-order with a 1/1/2/4/4/4-ktile ramp for ch0);
x slabs, bias rows and output tiles ride the scalar ring (8.5 MB).

bias (pre-divided by s_w) enters the PSUM accumulation as a K=1 matmul
of ones[1,128] x biasq[1,512] issued between k-tiles 14 and 15.
Eviction is split: VectorE evicts batch-tile 0 with a fused
tensor_scalar max(acc*s_w, 0), ScalarE evicts batch-tile 1 with
activation Relu(scale=s_w). Outputs leave as fp16.

Caveat from prior tuning: the chip's power manager can drop the PE to
2.0 GHz under sustained load (flat 454 ns vs 379 ns matmuls in the
profile) - compare runs only at equal clock.
"""

import os
import sys

for _p in ("/opt/trn_rl_repo", "/root/.axon_site/_ro/trn_rl_repo"):
    if os.path.isdir(_p) and _p not in sys.path:
        sys.path.insert(0, _p)

import numpy as np
import ml_dtypes

import concourse.bacc as bacc
import concourse.mybir as mybir
from concourse import tile
from concourse.bass_utils import run_bass_kernel_spmd

B, S, C, E = 256, 2048, 32, 2048
NCORES = 8
CPC = C // NCORES          # channels per core = 4
P = 128
KT = S // P                # 16 k-tiles
NBT = B // P               # 2 batch tiles
FREE = 512                 # matmul moving free dim (one PSUM bank of f32)
NET = E // FREE            # 4 e-tiles
FP8_MAX = 15.0             # e3m4 scale target (max normal 15.5)
WRAMP = [1, 1, 2, 2, 2]    # ch0 W k0-7 piece sizes (k-tiles) on sync;
                           # x k8-15 rides sync next, then W k8-11/k12-15
XRAMP = [1, 1, 2, 4]       # ch0 x k0-7 piece sizes (k-tiles) on scalar
NWARM = 9                  # HAM warmup matmuls: full K=128 N=512 (K=1 MMs
                           # do NOT register as PE-busy for the HAM clock
                           # gate - measured). 9 x 427ns cold bridges
                           # ~7.0us to first-data (~11us)

_nc_cache = {}


def _build(s_w: float):
    nc = bacc.Bacc(None, target_bir_lowering=False)
    xt = nc.dram_tensor("xt", [CPC, P, KT, B], mybir.dt.float16, kind="ExternalInput")
    # W fp8e3, host-layouted [c, partition, ktile, e]: a k-range DMA reads
    # nkt*E contiguous bytes per partition (>=2 KB for nkt>=1).
    w8 = nc.dram_tensor("w8", [CPC, P, KT, E], mybir.dt.float8e3, kind="ExternalInput")
    biasq = nc.dram_tensor("biasq", [CPC, E], mybir.dt.float16, kind="ExternalInput")
    out = nc.dram_tensor("out", [B, CPC, E], mybir.dt.float16, kind="ExternalOutput")

    with tile.TileContext(nc) as tc:
        with (
            tc.tile_pool(name="const", bufs=1) as const,
            tc.tile_pool(name="xpool", bufs=2) as xpool,
            tc.tile_pool(name="bqpool", bufs=CPC) as bqpool,
            tc.tile_pool(name="wpool", bufs=3) as wpool,
            tc.tile_pool(name="opool", bufs=4) as opool,
            tc.tile_pool(name="psum", bufs=NBT * NET, space="PSUM") as psum,
        ):
            zbias = const.tile([P, 1], mybir.dt.float32, name="zbias", tag="zb")
            nc.any.memset(zbias[:], 0.0)
            ones = const.tile([1, P], mybir.dt.float16, name="ones", tag="ones")
            nc.any.memset(ones[:], 1.0)
            wrm = const.tile([P, FREE], mybir.dt.float16, name="wrm", tag="wrm")
            nc.any.memset(wrm[:], 1.0)

            # HAM warmup: throwaway FULL K=128 N=512 matmuls keep the PE
            # genuinely busy early so the clock gate is open when the
            # first real matmul lands
            psw = psum.tile([P, FREE], mybir.dt.float32, name="psw", tag="ps")
            for _ in range(NWARM):
                nc.tensor.matmul(psw[:], wrm[:, :P], wrm[:], start=True, stop=True)

            # ---- front-loaded critical DMAs ----
            # SDMA engines round-robin per packet across rings, so the
            # latency-critical W stream rides the sync ring ALONE in
            # need-order; everything else (x slabs, bias, outputs) rides
            # the scalar ring.
            wsb0 = wpool.tile([P, KT, E], mybir.dt.float8e3, name="wsb", tag="wsb")
            xsb0 = xpool.tile([P, KT, B], mybir.dt.float16, name="xsb")
            k0 = 0
            for nkt in WRAMP:
                nc.sync.dma_start(wsb0[:, k0 : k0 + nkt, :], w8[0, :, k0 : k0 + nkt, :])
                k0 += nkt
            # x k8-15 is needed ~14us after W k8-15 but is small (512 KB);
            # slotting it here on the sync FIFO keeps the scalar ring's
            # early load down to x k0-7 + bias (the fabric saturates at
            # ~435 GB/s in the first 10us otherwise)
            nc.sync.dma_start(xsb0[:, 8:, :], xt[0, :, 8:, :])
            nc.sync.dma_start(wsb0[:, 8:12, :], w8[0, :, 8:12, :])
            nc.sync.dma_start(wsb0[:, 12:, :], w8[0, :, 12:, :])
            k0 = 0
            for nkt in XRAMP:
                nc.scalar.dma_start(xsb0[:, k0 : k0 + nkt, :], xt[0, :, k0 : k0 + nkt, :])
                k0 += nkt
            bqtiles = []
            for c in range(CPC):
                bq = bqpool.tile([1, E], mybir.dt.float16, name="bq", tag="bq")
                nc.scalar.dma_start(bq[:], biasq[c : c + 1, :])
                bqtiles.append(bq)

            xtiles = {0: xsb0}
            wtiles = {0: wsb0}

            def prefetch_w(c):
                wsb = wpool.tile([P, KT, E], mybir.dt.float8e3, name="wsb", tag="wsb")
                for g in range(4):
                    nc.sync.dma_start(
                        wsb[:, g * 4 : (g + 1) * 4, :], w8[c, :, g * 4 : (g + 1) * 4, :]
                    )
                wtiles[c] = wsb

            def prefetch_x(c):
                # deferred to k==8 so the 2 MB x slab does not steal early
                # SDMA share from the current channel's critical W pieces
                xsb = xpool.tile([P, KT, B], mybir.dt.float16, name="xsb")
                nc.scalar.dma_start(xsb[:], xt[c, :, :, :])
                xtiles[c] = xsb

            def evict(c, bt, et, src, dst):
                # DVE takes batch-tile 0 (fused max(acc*s_w, 0)), ScalarE
                # takes batch-tile 1 (Relu activation, scale=s_w)
                if bt == 0:
                    nc.vector.tensor_scalar(
                        dst,
                        src,
                        s_w,
                        0.0,
                        mybir.AluOpType.mult,
                        mybir.AluOpType.max,
                    )
                else:
                    nc.scalar.activation(
                        dst,
                        src,
                        mybir.ActivationFunctionType.Relu,
                        bias=zbias[:],
                        scale=s_w,
                    )

            for c in range(CPC - 1):
                xsb = xtiles[c]
                wsb = wtiles[c]
                ps = [
                    [
                        psum.tile([P, FREE], mybir.dt.float32, name="ps", tag="ps")
                        for _ in range(NET)
                    ]
                    for _ in range(NBT)
                ]
                bq = bqtiles[c]
                for k in range(KT):
                    if k == KT - 1:
                        # bias joins the accumulation here: K=1 matmul of
                        # ones[1,128] x biasq[1,512]; deps long resolved
                        for bt in range(NBT):
                            for et in range(NET):
                                nc.tensor.matmul(
                                    ps[bt][et][:],
                                    ones[:],
                                    bq[:, et * FREE : (et + 1) * FREE],
                                    start=False,
                                    stop=False,
                                )
                    for bt in range(NBT):
                        lhsT = xsb[:, k, bt * P : (bt + 1) * P]
                        for et in range(NET):
                            nc.tensor.matmul(
                                ps[bt][et][:],
                                lhsT,
                                wsb[:, k, et * FREE : (et + 1) * FREE],
                                start=(k == 0),
                                stop=(k == KT - 1),
                            )
                    if k == 0:
                        prefetch_w(c + 1)
                    if k == 8:
                        prefetch_x(c + 1)

                for bt in range(NBT):
                    ot = opool.tile([P, E], mybir.dt.float16)
                    for et in range(NET):
                        evict(c, bt, et, ps[bt][et][:], ot[:, et * FREE : (et + 1) * FREE])
                    nc.scalar.dma_start(out[bt * P : (bt + 1) * P, c, :], ot[:])

            # Last channel runs per-PSUM-bank so banks close (and evict +
            # store) one at a time instead of all 8 at the kernel tail.
            # LDWEIGHTS per matmul (145 ns) still hides under the 216 ns
            # N=512 stream via the PE reorder window.
            c = CPC - 1
            xsb = xtiles[c]
            wsb = wtiles[c]
            bq = bqtiles[c]
            for bt in range(NBT):
                ot = opool.tile([P, E], mybir.dt.float16)
                for et in range(NET):
                    psb = psum.tile([P, FREE], mybir.dt.float32, name="ps", tag="ps")
                    for k in range(KT):
                        if k == KT - 1:
                            nc.tensor.matmul(
                                psb[:],
                                ones[:],
                                bq[:, et * FREE : (et + 1) * FREE],
                                start=False,
                                stop=False,
                            )
                        nc.tensor.matmul(
                            psb[:],
                            xsb[:, k, bt * P : (bt + 1) * P],
                            wsb[:, k, et * FREE : (et + 1) * FREE],
                            start=(k == 0),
                            stop=(k == KT - 1),
                        )
                    dst = ot[:, et * FREE : (et + 1) * FREE]
                    evict(c, bt, et, psb[:], dst)
                    # sync ring is idle by now (W stream done); split the
                    # final stores across both rings to shorten the tail
                    oeng = nc.sync if bt == 0 else nc.scalar
                    oeng.dma_start(
                        out[bt * P : (bt + 1) * P, c, et * FREE : (et + 1) * FREE],
                        dst,
                    )
    nc.compile()
    return nc


def _get_nc(s_w: float):
    key = round(float(s_w), 12)
    if key not in _nc_cache:
        _nc_cache[key] = _build(float(s_w))
    return _nc_cache[key]


def _run(x, W, b, **spmd_kwargs):
    s_w = float(np.abs(W).max() / FP8_MAX)
    nc = _get_nc(s_w)

    W8 = (W * (1.0 / s_w)).astype(ml_dtypes.float8_e3m4)

    in_maps = []
    for i in range(NCORES):
        c0, c1 = i * CPC, (i + 1) * CPC
        # x[:, :, c] -> [CPC, P, KT, B]: s = k*P + p
        xt_i = np.ascontiguousarray(
            x[:, :, c0:c1]
            .transpose(2, 1, 0)
            .reshape(CPC, KT, P, B)
            .transpose(0, 2, 1, 3)
            .astype(np.float16)
        )
        # [CPC, S, E] -> [CPC, P, KT, E] with s = k*P + p
        w8_i = np.ascontiguousarray(
            W8[c0:c1].reshape(CPC, KT, P, E).transpose(0, 2, 1, 3)
        )
        biasq_i = np.ascontiguousarray((b[c0:c1] / s_w).astype(np.float16))
        in_maps.append({"xt": xt_i, "w8": w8_i, "biasq": biasq_i})

    res = run_bass_kernel_spmd(nc, in_maps, core_ids=list(range(NCORES)), **spmd_kwargs)
    out = np.concatenate(
        [r["out"].astype(np.float32) for r in res.results], axis=1
    )
    return out, res


def kernel(x: np.ndarray, W: np.ndarray, b: np.ndarray) -> np.ndarray:
    out, _ = _run(x, W, b)
    return out


# revision 11
# speedup vs baseline: 1.0464x; 1.0464x over previous
"""Grouped per-channel Linear + ReLU on 8 TRN2 NeuronCores.

Problem: out[b,c,e] = relu(sum_s x[b,s,c] * W[c,s,e] + bias[c,e])
  x: (256, 2048, 32) f32, W: (32, 2048, 2048) f32, bias: (32, 2048) f32
  out: (256, 32, 2048) f32

Sharding: expert/channel parallel - core i computes channels [4i, 4i+4).
Each core runs 4 independent GEMMs of (256x2048)@(2048x2048) with the
contraction dim S on SBUF partitions; x is host-transposed to
[CPC, P, KT, B] fp16 so DMA descriptors are >=2 KB contiguous runs.

Quantization (rel l2 ~1.8e-2, gate 2e-2, matches numpy sim to <1%):
  - k-tiles 0-13: W in float8e3 (E3M4, 4 mantissa bits, scaled to max
    15.0) streamed from HBM straight into the PE as the moving operand
    of an fp16(x) x fp8e3(W) matmul - no on-chip dequant at all, and
    fp8 at normal rate costs the same N cycles/matmul as fp16.
  - k-tiles 14-15: both operands float8e4 with perf_mode=DoubleRow -
    one matmul contracts 256 rows in the same 216 ns, saving ~7us of
    PE time per core. Scales: x/s_x (s_x=|x|max/224) and W*s_x/s_w so
    the product lands in the same 1/s_w units as the main stream.

DMA: W rides the sync ring alone, in k-order, ramped 1/1/2/2/2/4/2
k-tile pieces for ch0 so completion semaphores pace the warm PE; the
scalar ring carries x k0-7 + bias early, with x k8-15 dep-gated behind
the k2 matmuls (the first ~15us saturate the shared HBM stack at ~435
GB/s/core; un-gated it starves the sibling core's W stream).

HAM: 9 throwaway FULL K=128 N=512 warmup matmuls (K=1 matmuls do NOT
register as PE-busy for the clock gate - measured) bridge ~6.8us to
first-data (~10us) so real matmuls start at 2.4 GHz.

bias (pre-divided by s_w) joins the PSUM accumulation as a K=1 matmul
of ones[1,128] x biasq[1,512] after k13. Eviction is split: VectorE
evicts batch-tile 0 with a fused tensor_scalar max(acc*s_w, 0),
ScalarE evicts batch-tile 1 with activation Relu(scale=s_w). The last
channel runs per-PSUM-bank (own k-loop per bank) so banks close one at
a time and the final eviction+store exposure is ~1 bank, with stores
split across both rings. Outputs leave as fp16.
"""

import os
import sys

for _p in ("/opt/trn_rl_repo", "/root/.axon_site/_ro/trn_rl_repo"):
    if os.path.isdir(_p) and _p not in sys.path:
        sys.path.insert(0, _p)

import numpy as np
import ml_dtypes

import concourse.bacc as bacc
import concourse.mybir as mybir
from concourse import tile
from concourse.bass_utils import run_bass_kernel_spmd
from concourse.tile_rust import add_dep_helper

B, S, C, E = 256, 2048, 32, 2048
NCORES = 8
CPC = C // NCORES          # channels per core = 4
P = 128
KT = S // P                # 16 k-tiles
KTN = KT - 2               # 14 k-tiles on the normal fp16 x fp8e3 path
NBT = B // P               # 2 batch tiles
FREE = 512                 # matmul moving free dim (one PSUM bank of f32)
NET = E // FREE            # 4 e-tiles
FP8_MAX = 15.0             # e3m4 scale target (max normal 15.5)
X8_MAX = 224.0             # e4m3 scale target (TRN max normal 240)
WRAMP = [1, 1, 2, 2, 2, 4, 2]  # ch0 W piece sizes (k-tiles) on sync
XRAMP = [1, 1, 2, 4]       # ch0 x k0-7 piece sizes (k-tiles) on scalar
NWARM = 9                  # HAM warmup matmuls: full K=128 N=512 (K=1 MMs
                           # do NOT register as PE-busy for the clock
                           # gate - measured). 9 x 427ns cold bridges
                           # ~6.8us to first-data (~10us)

_nc_cache = {}


def _build(s_w: float):
    nc = bacc.Bacc(None, target_bir_lowering=False)
    xt = nc.dram_tensor("xt", [CPC, P, KT, B], mybir.dt.float16, kind="ExternalInput")
    # W fp8e3, host-layouted [c, partition, ktile, e]: a k-range DMA reads
    # nkt*E contiguous bytes per partition (>=2 KB for nkt>=1).
    w8 = nc.dram_tensor("w8", [CPC, P, KTN, E], mybir.dt.float8e3, kind="ExternalInput")
    # DoubleRow pair (k-tiles 14-15), both operands e4m3
    w4 = nc.dram_tensor("w4", [CPC, P, 2, E], mybir.dt.float8e4, kind="ExternalInput")
    x4 = nc.dram_tensor("x4", [CPC, P, 2, B], mybir.dt.float8e4, kind="ExternalInput")
    biasq = nc.dram_tensor("biasq", [CPC, E], mybir.dt.float16, kind="ExternalInput")
    out = nc.dram_tensor("out", [B, CPC, E], mybir.dt.float16, kind="ExternalOutput")

    with tile.TileContext(nc) as tc:
        with (
            tc.tile_pool(name="const", bufs=1) as const,
            tc.tile_pool(name="xpool", bufs=2) as xpool,
            tc.tile_pool(name="x4pool", bufs=2) as x4pool,
            tc.tile_pool(name="bqpool", bufs=CPC) as bqpool,
            tc.tile_pool(name="wpool", bufs=3) as wpool,
            tc.tile_pool(name="w4pool", bufs=2) as w4pool,
            tc.tile_pool(name="opool", bufs=4) as opool,
            tc.tile_pool(name="psum", bufs=NBT * NET, space="PSUM") as psum,
        ):
            zbias = const.tile([P, 1], mybir.dt.float32, name="zbias", tag="zb")
            nc.any.memset(zbias[:], 0.0)
            ones = const.tile([1, P], mybir.dt.float16, name="ones", tag="ones")
            nc.any.memset(ones[:], 1.0)
            wrm = const.tile([P, FREE], mybir.dt.float16, name="wrm", tag="wrm")
            nc.any.memset(wrm[:], 1.0)

            # HAM warmup: throwaway FULL K=128 N=512 matmuls keep the PE
            # genuinely busy early so the clock gate is open when the
            # first real matmul lands
            psw = psum.tile([P, FREE], mybir.dt.float32, name="psw", tag="ps")
            for _ in range(NWARM):
                nc.tensor.matmul(psw[:], wrm[:, :P], wrm[:], start=True, stop=True)

            # ---- front-loaded critical DMAs ----
            # SDMA engines round-robin per packet across rings; the
            # latency-critical W stream rides the sync ring ALONE in
            # need-order; x slabs, bias and outputs ride the scalar ring.
            wsb0 = wpool.tile([P, KTN, E], mybir.dt.float8e3, name="wsb", tag="wsb")
            w4sb0 = w4pool.tile([P, 2, E], mybir.dt.float8e4, name="w4sb", tag="w4sb")
            xsb0 = xpool.tile([P, KT, B], mybir.dt.float16, name="xsb")
            x4sb0 = x4pool.tile([P, 2, B], mybir.dt.float8e4, name="x4sb", tag="x4sb")
            k0 = 0
            for nkt in WRAMP:
                nc.sync.dma_start(wsb0[:, k0 : k0 + nkt, :], w8[0, :, k0 : k0 + nkt, :])
                k0 += nkt
            nc.sync.dma_start(w4sb0[:], w4[0, :, :, :])
            k0 = 0
            for nkt in XRAMP:
                nc.scalar.dma_start(xsb0[:, k0 : k0 + nkt, :], xt[0, :, k0 : k0 + nkt, :])
                k0 += nkt
            nc.scalar.dma_start(x4sb0[:], x4[0, :, :, :])
            bqtiles = []
            for c in range(CPC):
                bq = bqpool.tile([1, E], mybir.dt.float16, name="bq", tag="bq")
                nc.scalar.dma_start(bq[:], biasq[c : c + 1, :])
                bqtiles.append(bq)

            xtiles = {0: xsb0}
            wtiles = {0: (wsb0, w4sb0)}
            x4tiles = {0: x4sb0}

            def prefetch_w(c):
                wsb = wpool.tile([P, KTN, E], mybir.dt.float8e3, name="wsb", tag="wsb")
                for g in range(3):
                    nc.sync.dma_start(
                        wsb[:, g * 4 : (g + 1) * 4, :], w8[c, :, g * 4 : (g + 1) * 4, :]
                    )
                nc.sync.dma_start(wsb[:, 12:, :], w8[c, :, 12:, :])
                w4sb = w4pool.tile([P, 2, E], mybir.dt.float8e4, name="w4sb", tag="w4sb")
                nc.sync.dma_start(w4sb[:], w4[c, :, :, :])
                wtiles[c] = (wsb, w4sb)

            def prefetch_x(c):
                # deferred to k==8 so the 2 MB x slab does not steal early
                # SDMA share from the current channel's critical W pieces
                xsb = xpool.tile([P, KT, B], mybir.dt.float16, name="xsb")
                nc.scalar.dma_start(xsb[:], xt[c, :, :, :])
                xtiles[c] = xsb
                x4sb = x4pool.tile([P, 2, B], mybir.dt.float8e4, name="x4sb", tag="x4sb")
                nc.scalar.dma_start(x4sb[:], x4[c, :, :, :])
                x4tiles[c] = x4sb

            def evict(bt, src, dst):
                # DVE takes batch-tile 0 (fused max(acc*s_w, 0)), ScalarE
                # takes batch-tile 1 (Relu activation, scale=s_w)
                if bt == 0:
                    nc.vector.tensor_scalar(
                        dst,
                        src,
                        s_w,
                        0.0,
                        mybir.AluOpType.mult,
                        mybir.AluOpType.max,
                    )
                else:
                    nc.scalar.activation(
                        dst,
                        src,
                        mybir.ActivationFunctionType.Relu,
                        bias=zbias[:],
                        scale=s_w,
                    )

            def bias_mm(ps, bq, et):
                nc.tensor.matmul(
                    ps,
                    ones[:],
                    bq[:, et * FREE : (et + 1) * FREE],
                    start=False,
                    stop=False,
                )

            def dr_mm(ps, x4sb, w4sb, bt, et):
                # k-tiles 14-15: e4m3 x e4m3 DoubleRow - contracts 256
                # rows in one 216 ns matmul and closes the group
                nc.tensor.matmul(
                    ps,
                    x4sb[:, :, bt * P : (bt + 1) * P],
                    w4sb[:, :, et * FREE : (et + 1) * FREE],
                    start=False,
                    stop=True,
                    perf_mode=mybir.MatmulPerfMode.DoubleRow,
                )

            for c in range(CPC - 1):
                xsb = xtiles[c]
                wsb, w4sb = wtiles[c]
                x4sb = x4tiles[c]
                ps = [
                    [
                        psum.tile([P, FREE], mybir.dt.float32, name="ps", tag="ps")
                        for _ in range(NET)
                    ]
                    for _ in range(NBT)
                ]
                bq = bqtiles[c]
                for k in range(KTN):
                    for bt in range(NBT):
                        lhsT = xsb[:, k, bt * P : (bt + 1) * P]
                        for et in range(NET):
                            mm = nc.tensor.matmul(
                                ps[bt][et][:],
                                lhsT,
                                wsb[:, k, et * FREE : (et + 1) * FREE],
                                start=(k == 0),
                                stop=False,
                            )
                    if c == 0 and k == 2:
                        gate_mm = mm
                    if c == 0 and k == 3:
                        # x k8-15 gated past the k2 matmuls: the early
                        # window saturates the HBM stack shared with the
                        # sibling core, so this 512 KB must not compete
                        xd = nc.scalar.dma_start(xsb0[:, 8:, :], xt[0, :, 8:, :])
                        add_dep_helper(
                            xd.ins, gate_mm.ins, reason="x tail after k2 MMs"
                        )
                    if k == 0:
                        prefetch_w(c + 1)
                    if k == 8:
                        prefetch_x(c + 1)

                for bt in range(NBT):
                    for et in range(NET):
                        bias_mm(ps[bt][et][:], bq, et)
                    for et in range(NET):
                        dr_mm(ps[bt][et][:], x4sb, w4sb, bt, et)

                for bt in range(NBT):
                    ot = opool.tile([P, E], mybir.dt.float16)
                    for et in range(NET):
                        evict(bt, ps[bt][et][:], ot[:, et * FREE : (et + 1) * FREE])
                    nc.scalar.dma_start(out[bt * P : (bt + 1) * P, c, :], ot[:])

            # Last channel runs per-PSUM-bank so banks close (and evict +
            # store) one at a time instead of all 8 at the kernel tail.
            c = CPC - 1
            xsb = xtiles[c]
            wsb, w4sb = wtiles[c]
            x4sb = x4tiles[c]
            bq = bqtiles[c]
            for bt in range(NBT):
                ot = opool.tile([P, E], mybir.dt.float16)
                for et in range(NET):
                    psb = psum.tile([P, FREE], mybir.dt.float32, name="ps", tag="ps")
                    for k in range(KTN):
                        nc.tensor.matmul(
                            psb[:],
                            xsb[:, k, bt * P : (bt + 1) * P],
                            wsb[:, k, et * FREE : (et + 1) * FREE],
                            start=(k == 0),
                            stop=False,
                        )
                    bias_mm(psb[:], bq, et)
                    dr_mm(psb[:], x4sb, w4sb, bt, et)
                    dst = ot[:, et * FREE : (et + 1) * FREE]
                    evict(bt, psb[:], dst)
                    # sync ring is idle by now (W stream done); split the
                    # final stores across both rings to shorten the tail
                    oeng = nc.sync if bt == 0 else nc.scalar
                    oeng.dma_start(
                        out[bt * P : (bt + 1) * P, c, et * FREE : (et + 1) * FREE],
                        dst,
                    )
    nc.compile()
    return nc


def _get_nc(s_w: float):
    key = round(float(s_w), 12)
    if key not in _nc_cache:
        _nc_cache[key] = _build(float(s_w))
    return _nc_cache[key]


def _run(x, W, b, **spmd_kwargs):
    s_w = float(np.abs(W).max() / FP8_MAX)
    s_x = float(np.abs(x).max() / X8_MAX)
    nc = _get_nc(s_w)

    SDR = KTN * P  # first contraction row of the DoubleRow pair
    W8 = (W[:, :SDR, :] * (1.0 / s_w)).astype(ml_dtypes.float8_e3m4)
    W4 = (W[:, SDR:, :] * (s_x / s_w)).astype(ml_dtypes.float8_e4m3)

    in_maps = []
    for i in range(NCORES):
        c0, c1 = i * CPC, (i + 1) * CPC
        # x[:, :, c] -> [CPC, P, KT, B]: s = k*P + p
        xc = x[:, :, c0:c1].transpose(2, 1, 0)  # (CPC, S, B)
        xt_i = np.ascontiguousarray(
            xc.reshape(CPC, KT, P, B).transpose(0, 2, 1, 3).astype(np.float16)
        )
        x4_i = np.ascontiguousarray(
            (xc[:, SDR:, :] * (1.0 / s_x))
            .reshape(CPC, 2, P, B)
            .transpose(0, 2, 1, 3)
            .astype(ml_dtypes.float8_e4m3)
        )
        # [CPC, S', E] -> [CPC, P, kt, E] with s = k*P + p
        w8_i = np.ascontiguousarray(
            W8[c0:c1].reshape(CPC, KTN, P, E).transpose(0, 2, 1, 3)
        )
        w4_i = np.ascontiguousarray(
            W4[c0:c1].reshape(CPC, 2, P, E).transpose(0, 2, 1, 3)
        )
        biasq_i = np.ascontiguousarray((b[c0:c1] / s_w).astype(np.float16))
        in_maps.append(
            {"xt": xt_i, "w8": w8_i, "w4": w4_i, "x4": x4_i, "biasq": biasq_i}
        )

    res = run_bass_kernel_spmd(nc, in_maps, core_ids=list(range(NCORES)), **spmd_kwargs)
    out = np.concatenate(
        [r["out"].astype(np.float32) for r in res.results], axis=1
    )
    return out, res


def kernel(x: np.ndarray, W: np.ndarray, b: np.ndarray) -> np.ndarray:
    out, _ = _run(x, W, b)
    return out
